# revision 11
# baseline (speedup 1.0000x reference)
"""MLA QKV projection kernel for Trainium2 (8 NeuronCores, Bass/Tile).

Computes the DeepSeek-MLA q/k/v projection:
  q  = rmsnorm(hs @ q_a_w.T) @ q_b_w.T          -> [b, H, s, 192]  (pe cols rope-interleaved)
  ckv = hs @ kv_a_w.T ; compressed, k_pe = split
  kv = rmsnorm(compressed) @ kv_b_w.T           -> k_nope, v
  out = concat([q, concat(k_nope, k_pe), pad(v)], head axis)  -> [b, 3H, s, 192]

Strategy: pure data-parallel over tokens (8192 tokens -> 1024/core); weights
replicated, loaded from HBM exactly once per core. GEMMs run bf16 except the
kv-B projection (fp8e4 DoubleRow, K-paired: 2x rate) and the first 512 of the
q-B projection's K (also fp8e4 DoubleRow). The fp8/bf16 parts accumulate in a
common 2^14-scaled PSUM domain (acts x16 fp8 | weights x1024 fp8 | bf16
weights x16384), so every B eviction is one uniform 1/16384-scaled copy.
RMSNorm is applied by rescaling the A-GEMM output in SBUF (per-token scale =
per-partition broadcast); layernorm gammas and the rope interleave permutation
are folded into the weights on the host. Matmuls are grouped 4-consecutive
per stationary tile (512-col chunks of a [128,2048] PSUM tile) to amortize
the ~200ns LDWEIGHTS cost of switching the stationary operand. PSUM
evictions are split across the scalar/vector/pool engines and B weights are
prefetched a chunk-group ahead on a loads-only DMA queue so the PE never
stalls on PSUM rotation or weight arrival.
"""

import sys
import types

import numpy as np

# ---- constants (hardcoded problem shape) ----
H = 32
D_NOPE = 128
D_ROPE = 64
D_Q = 192
D_V = 128
R_KV = 512
RQ = 1536
DMODEL = 4096
EPS = 1e-6
B, S = 2, 4096
NTOK = B * S            # 8192
NCORES = 8
TPC = NTOK // NCORES    # 1024 tokens per core
KT = DMODEL // 128      # 32 k-tiles for the A GEMM
KSEG = 4                # A GEMM k-segments (SBUF-accumulated)
KPS = KT // KSEG        # 8 k-tiles per segment
ACOLS = RQ + R_KV       # 2048 (kpe's 64 cols handled in a separate pass)
QOUT = H * D_Q          # 6144
KVOUT = H * (D_NOPE + D_V)  # 8192

Q8_J = 2                # q-B: ktile pairs 0..Q8_J*2-1 in fp8 DR (512 of K)
QB_K = 12 - 2 * Q8_J    # remaining q-B bf16 ktiles
SC_ACT = 16.0           # fp8 activation scale
SC_W = 1024.0           # fp8 weight scale
SC_PSUM = SC_ACT * SC_W  # 16384 = common B psum domain


def _ensure_env():
    for p in ("/opt/trn_rl_repo", "/root/.axon_site"):
        if p not in sys.path:
            sys.path.insert(0, p)
    if "antenv.axon_hooks" not in sys.modules:
        try:
            import antenv  # noqa: F401
            import antenv.axon_hooks  # noqa: F401
        except ImportError:
            mod = types.ModuleType("antenv.axon_hooks")
            mod._hook = None
            mod.set_axon_ntff_profile_hook = lambda h: setattr(mod, "_hook", h)
            mod.get_axon_ntff_profile_hook = lambda: mod._hook
            sys.modules["antenv.axon_hooks"] = mod
            try:
                import antenv
                antenv.axon_hooks = mod
            except ImportError:
                pass


def _perm64():
    # inverse view of x.reshape(32,2).swapaxes ->  y[k] = x[2*(k%32) + k//32]
    return np.array([2 * (k % 32) + k // 32 for k in range(64)], dtype=np.int64)


_CACHE = {}


def _build():
    if "nc" in _CACHE:
        return _CACHE["nc"]
    _ensure_env()
    from concourse import bacc
    import concourse.mybir as mybir
    import concourse.tile as tile
    from concourse.masks import make_identity

    F32 = mybir.dt.float32
    BF16 = mybir.dt.bfloat16
    FP8 = mybir.dt.float8e4
    DR = mybir.MatmulPerfMode.DoubleRow
    AF = mybir.ActivationFunctionType
    ALU = mybir.AluOpType

    nc = bacc.Bacc("TRN2", target_bir_lowering=False, debug=False)
    hsT_d = nc.dram_tensor("hsT", [DMODEL, TPC], BF16, kind="ExternalInput")
    waT_d = nc.dram_tensor("waT", [DMODEL, ACOLS + D_ROPE], BF16, kind="ExternalInput")
    qb8_d = nc.dram_tensor("qb8", [Q8_J, 128, 2, QOUT], FP8, kind="ExternalInput")
    qbb_d = nc.dram_tensor("qbb", [QB_K * 128, QOUT], BF16, kind="ExternalInput")
    kv8_d = nc.dram_tensor("kv8", [2, 128, 2, KVOUT], FP8, kind="ExternalInput")
    out_d = nc.dram_tensor("out", [3 * H, TPC, D_Q], BF16, kind="ExternalOutput")

    def outv(h, mp, d0, w, n_m=2):
        # DRAM view [p, mi, w] for tokens mp*256..(mp+1)*256, head h cols d0:d0+w
        return out_d[h, mp * 128 * n_m:(mp + 1) * 128 * n_m, d0:d0 + w].rearrange(
            "(mi p) w -> p mi w", p=128)

    HALF = ACOLS // 2

    with tile.TileContext(nc) as tc:
        with tc.tile_pool(name="persist", bufs=1) as persist:
            kpe_sb = persist.tile([128, 8, D_ROPE], BF16)
            stats = persist.tile([128, 8, 2], F32)
            s_q = persist.tile([128, 8], F32)
            s_kv = persist.tile([128, 8], F32)
            at_q8 = persist.tile([128, Q8_J, 2, 8, 128], FP8)   # 4KB/p
            at_qb = persist.tile([128, QB_K, 8, 128], BF16)     # 16KB/p
            at_kv8 = persist.tile([128, 2, 2, 8, 128], FP8)     # 4KB/p
            sconst = persist.tile([128, 1], F32)
            ident = persist.tile([128, 128], F32)
            identb = persist.tile([128, 128], BF16)
            nc.gpsimd.memset(sconst, 1.0 / SC_PSUM)
            make_identity(nc, ident)
            nc.vector.tensor_copy(identb, ident)

            # ---------------- phases A + T (a_sb-scoped) ----------------
            with tc.tile_pool(name="aph", bufs=1) as aph:
                a_sb = aph.tile([128, 8, ACOLS], BF16)          # 32KB/p token-major A out
                kpeT = aph.tile([64, TPC], BF16)
                scr = aph.tile([128, RQ], F32)                  # square scratch
                scrb = aph.tile([128, 2, HALF], BF16)           # psum-evict staging

                with tc.tile_pool(name="hst", bufs=1) as hstp:
                    hst = [hstp.tile([128, TPC], BF16, name=f"hst{k}", tag=f"hst{k}")
                           for k in range(KT)]
                    for k in range(KT):
                        eng = nc.scalar if k % 2 == 0 else nc.gpsimd
                        eng.dma_start(out=hst[k], in_=hsT_d[k * 128:(k + 1) * 128, :])

                    with tc.tile_pool(name="wa", bufs=KPS + 2) as wap, \
                         tc.tile_pool(name="scl", bufs=4) as sclp, \
                         tc.tile_pool(name="psA", bufs=2, space="PSUM") as psA:
                        for kseg in range(KSEG):
                            wts = []
                            for kk in range(KPS):
                                k = kseg * KPS + kk
                                t = wap.tile([128, ACOLS], BF16, tag="wa")
                                nc.sync.dma_start(out=t, in_=waT_d[k * 128:(k + 1) * 128, 0:ACOLS])
                                wts.append(t)
                            for mp in range(4):
                                pss = [psA.tile([128, ACOLS], F32, tag="psA", name=f"psA{i}")
                                       for i in range(2)]
                                for kk in range(KPS):
                                    k = kseg * KPS + kk
                                    for mi in range(2):
                                        m = 2 * mp + mi
                                        for c in range(4):
                                            nc.tensor.matmul(
                                                pss[mi][:, c * 512:(c + 1) * 512],
                                                hst[k][:, m * 128:(m + 1) * 128],
                                                wts[kk][:, c * 512:(c + 1) * 512],
                                                start=(kk == 0), stop=(kk == KPS - 1))
                                for mi in range(2):
                                    m = 2 * mp + mi
                                    lo = a_sb[:, m, 0:HALF]
                                    hi = a_sb[:, m, HALF:ACOLS]
                                    if kseg == 0:
                                        nc.scalar.activation(lo, pss[mi][:, 0:HALF], AF.Copy)
                                        nc.vector.tensor_copy(hi, pss[mi][:, HALF:ACOLS])
                                    else:
                                        # DVE adds low half from PSUM; scalar stages the
                                        # high half to SBUF (fast PSUM release), pool adds it
                                        nc.vector.tensor_add(lo, lo, pss[mi][:, 0:HALF])
                                        nc.scalar.activation(scrb[:, mi, :], pss[mi][:, HALF:ACOLS], AF.Copy)
                                        nc.gpsimd.tensor_add(hi, hi, scrb[:, mi, :])
                                    if kseg == KSEG - 1:
                                        nc.scalar.activation(scr[:, 0:RQ], a_sb[:, m, 0:RQ],
                                                             AF.Square, accum_out=stats[:, m, 0:1])
                                        nc.scalar.activation(scr[:, 0:R_KV], a_sb[:, m, RQ:ACOLS],
                                                             AF.Square, accum_out=stats[:, m, 1:2])
                                if kseg == KSEG - 1:
                                    # per-pair rmsnorm scales + in-place rescale of a_sb
                                    for (dst, idx, dim) in ((s_q, 0, RQ), (s_kv, 1, R_KV)):
                                        t = sclp.tile([128, 2], F32, tag="sclt")
                                        nc.vector.tensor_scalar(
                                            out=t, in0=stats[:, 2 * mp:2 * mp + 2, idx],
                                            scalar1=1.0 / dim, scalar2=EPS,
                                            op0=ALU.mult, op1=ALU.add)
                                        nc.vector.reciprocal(t, t)
                                        nc.scalar.activation(dst[:, 2 * mp:2 * mp + 2], t, AF.Sqrt)
                                    for mi in range(2):
                                        m = 2 * mp + mi
                                        nc.scalar.activation(a_sb[:, m, 0:RQ], a_sb[:, m, 0:RQ],
                                                             AF.Copy, scale=s_q[:, m:m + 1])
                                        nc.vector.tensor_scalar_mul(
                                            a_sb[:, m, RQ:ACOLS], a_sb[:, m, RQ:ACOLS],
                                            s_kv[:, m:m + 1])

                    # ---- kpe pass: weights-stationary, output lands transposed ----
                    with tc.tile_pool(name="wk", bufs=4) as wkp, \
                         tc.tile_pool(name="psK", bufs=1, space="PSUM") as psKp:
                        psK = psKp.tile([64, TPC], F32)
                        for k in range(KT):
                            wk = wkp.tile([128, D_ROPE], BF16, tag="wk")
                            nc.sync.dma_start(out=wk, in_=waT_d[k * 128:(k + 1) * 128, ACOLS:ACOLS + D_ROPE])
                            for t2 in range(2):
                                nc.tensor.matmul(psK[:, t2 * 512:(t2 + 1) * 512], wk,
                                                 hst[k][:, t2 * 512:(t2 + 1) * 512],
                                                 start=(k == 0), stop=(k == KT - 1))
                        nc.vector.tensor_copy(kpeT, psK)

                # transpose kpe back to token-major; broadcast to all 32 key heads
                with tc.tile_pool(name="psKt", bufs=2, space="PSUM") as psKtp:
                    for m in range(8):
                        ptr = psKtp.tile([128, D_ROPE], BF16, tag="ptr")
                        nc.tensor.transpose(ptr, kpeT[:, m * 128:(m + 1) * 128], identb[0:64, 0:64])
                        nc.vector.tensor_copy(kpe_sb[:, m, :], ptr)
                for h in range(H):
                    nc.gpsimd.dma_start(out=outv(H + h, 0, D_NOPE, D_ROPE, n_m=8),
                                        in_=kpe_sb[:, :, :])

                # ---------------- phase T: transpose a_sb -> at tiles ----------------
                with tc.tile_pool(name="psT", bufs=4, space="PSUM") as psTp:
                    for f in range(16):
                        pt = psTp.tile([128, 8, 128], BF16, tag="pt")
                        for m in range(8):
                            nc.tensor.transpose(pt[:, m, :], a_sb[:, m, f * 128:(f + 1) * 128], identb)
                        if f < 2 * Q8_J:
                            nc.vector.tensor_scalar_mul(at_q8[:, f // 2, f % 2], pt, SC_ACT)
                        elif f < 12:
                            nc.vector.tensor_copy(at_qb[:, f - 2 * Q8_J], pt)
                        else:
                            nc.vector.tensor_scalar_mul(at_kv8[:, (f - 12) // 2, (f - 12) % 2], pt, SC_ACT)

            # ---------------- phase B: the two B GEMMs ----------------
            with tc.tile_pool(name="wq8", bufs=2 * Q8_J + 2) as wq8p, \
                 tc.tile_pool(name="wqb", bufs=2 * QB_K + 1) as wqbp, \
                 tc.tile_pool(name="kv8", bufs=6) as kv8p, \
                 tc.tile_pool(name="ev", bufs=3) as evp, \
                 tc.tile_pool(name="psB", bufs=2, space="PSUM") as psB:

                def load_q_w(cg):
                    w8ts, wbts = [], []
                    for j in range(Q8_J):
                        t = wq8p.tile([128, 2, 2048], FP8, tag="wq8", name=f"wq8_{cg}_{j}")
                        nc.sync.dma_start(out=t, in_=qb8_d[j, :, :, cg * 2048:(cg + 1) * 2048])
                        w8ts.append(t)
                    for kk in range(QB_K):
                        t = wqbp.tile([128, 2048], BF16, tag="wqb", name=f"wqb_{cg}_{kk}")
                        nc.sync.dma_start(out=t, in_=qbb_d[kk * 128:(kk + 1) * 128, cg * 2048:(cg + 1) * 2048])
                        wbts.append(t)
                    return w8ts, wbts

                def load_kv_w(cg):
                    kvts = []
                    for j in range(2):
                        t = kv8p.tile([128, 2, 2048], FP8, tag="kv8", name=f"kv8_{cg}_{j}")
                        nc.sync.dma_start(out=t, in_=kv8_d[j, :, :, cg * 2048:(cg + 1) * 2048])
                        kvts.append(t)
                    return kvts

                def b_evict_dma(pss, mp, cg, head_w, vsplit):
                    ev = evp.tile([128, 2, 2048], BF16, tag="ev")
                    # split each psum eviction across scalar+vector
                    nc.scalar.activation(ev[:, 0, 0:1024], pss[0][:, 0:1024],
                                         AF.Copy, scale=sconst[:, 0:1])
                    nc.vector.tensor_scalar_mul(ev[:, 0, 1024:2048], pss[0][:, 1024:2048],
                                                1.0 / SC_PSUM)
                    nc.vector.tensor_scalar_mul(ev[:, 1, 0:1024], pss[1][:, 0:1024],
                                                1.0 / SC_PSUM)
                    nc.scalar.activation(ev[:, 1, 1024:2048], pss[1][:, 1024:2048],
                                         AF.Copy, scale=sconst[:, 0:1])
                    col = cg * 2048
                    end = col + 2048
                    di = 0
                    while col < end:
                        h = col // head_w
                        seg = min(end, (h + 1) * head_w)
                        off = col - h * head_w
                        if not vsplit:
                            dst = outv(h, mp, off, seg - col)
                            eng = nc.gpsimd
                        elif off < D_NOPE:
                            seg = min(seg, h * head_w + D_NOPE)
                            dst = outv(H + h, mp, off, seg - col)
                            eng = nc.gpsimd if di % 2 == 0 else nc.scalar
                        else:
                            dst = outv(2 * H + h, mp, off - D_NOPE, seg - col)
                            eng = nc.gpsimd if di % 2 == 0 else nc.scalar
                        eng.dma_start(out=dst, in_=ev[:, :, col - cg * 2048:seg - cg * 2048])
                        di += 1
                        col = seg

                # B-q: 3 chunk-groups of 2048 cols, weights prefetched one cg ahead
                qw = {0: load_q_w(0)}
                kvw = {}
                for cg in range(3):
                    w8ts, wbts = qw.pop(cg)
                    for mp in range(4):
                        pss = [psB.tile([128, 2048], F32, tag="psB", name=f"psB{i}")
                               for i in range(2)]
                        for j in range(Q8_J):
                            for mi in range(2):
                                m = 2 * mp + mi
                                for c in range(4):
                                    nc.tensor.matmul(
                                        pss[mi][:, c * 512:(c + 1) * 512],
                                        at_q8[:, j, :, m, :],
                                        w8ts[j][:, :, c * 512:(c + 1) * 512],
                                        start=(j == 0), stop=False, perf_mode=DR)
                        for kk in range(QB_K):
                            for mi in range(2):
                                m = 2 * mp + mi
                                for c in range(4):
                                    nc.tensor.matmul(
                                        pss[mi][:, c * 512:(c + 1) * 512],
                                        at_qb[:, kk, m, :],
                                        wbts[kk][:, c * 512:(c + 1) * 512],
                                        start=False, stop=(kk == QB_K - 1))
                        if mp == 0:
                            if cg < 2:
                                qw[cg + 1] = load_q_w(cg + 1)
                            else:
                                kvw[0] = load_kv_w(0)
                                kvw[1] = load_kv_w(1)
                        b_evict_dma(pss, mp, cg, D_Q, False)

                # B-kv: 4 chunk-groups of 2048 cols (8 half-heads each)
                for cg in range(4):
                    kvts = kvw.pop(cg)
                    for mp in range(4):
                        pss = [psB.tile([128, 2048], F32, tag="psB", name=f"psB{i}")
                               for i in range(2)]
                        for j in range(2):
                            for mi in range(2):
                                m = 2 * mp + mi
                                for c in range(4):
                                    nc.tensor.matmul(
                                        pss[mi][:, c * 512:(c + 1) * 512],
                                        at_kv8[:, j, :, m, :],
                                        kvts[j][:, :, c * 512:(c + 1) * 512],
                                        start=(j == 0), stop=(j == 1),
                                        perf_mode=DR)
                        if mp == 0 and cg + 2 < 4:
                            kvw[cg + 2] = load_kv_w(cg + 2)
                        b_evict_dma(pss, mp, cg, D_NOPE + D_V, True)

    nc.compile()
    _CACHE["nc"] = nc
    return nc


def _prep_inputs(hidden_states, q_a_w, kv_a_w, q_b_w, kv_b_w, q_a_ln_w, kv_a_ln_w):
    import ml_dtypes
    F8 = ml_dtypes.float8_e4m3
    BF = ml_dtypes.bfloat16
    f32 = np.float32
    hs = np.asarray(hidden_states, dtype=f32).reshape(NTOK, DMODEL)
    hsT = np.ascontiguousarray(hs.T).astype(BF)            # [4096, 8192]
    perm = _perm64()

    q_a_w = np.asarray(q_a_w, dtype=f32)
    kv_a_w = np.asarray(kv_a_w, dtype=f32)
    kv_a_pe = kv_a_w[R_KV:][perm]                          # de-interleave k_pe rows
    wa = np.concatenate([q_a_w, kv_a_w[:R_KV], kv_a_pe], axis=0)   # [2112, 4096]
    waT = np.ascontiguousarray(wa.T).astype(BF)            # [4096, 2112]

    qb = np.asarray(q_b_w, dtype=f32) * np.asarray(q_a_ln_w, dtype=f32)[None, :]
    qb = qb.reshape(H, D_Q, RQ).copy()
    qb[:, D_NOPE:, :] = qb[:, D_NOPE + perm, :]            # de-interleave q_pe rows
    qbT = np.ascontiguousarray(qb.reshape(QOUT, RQ).T)     # [1536, 6144] f32

    # fp8 DR part: ktile pairs j -> planes i hold ktiles 2j+i, scaled x1024
    qb8 = np.empty((Q8_J, 128, 2, QOUT), dtype=F8)
    for j in range(Q8_J):
        for i in range(2):
            qb8[j, :, i, :] = (qbT[(2 * j + i) * 128:(2 * j + i + 1) * 128] * SC_W).astype(F8)
    # bf16 part scaled x16384 into the common psum domain
    qbb = np.ascontiguousarray(qbT[2 * Q8_J * 128:] * SC_PSUM).astype(BF)

    kvb = np.asarray(kv_b_w, dtype=f32) * np.asarray(kv_a_ln_w, dtype=f32)[None, :]
    kvbT = np.ascontiguousarray(kvb.T)                     # [512, 8192] f32
    kv8 = np.empty((2, 128, 2, KVOUT), dtype=F8)
    for j in range(2):
        for i in range(2):
            kv8[j, :, i, :] = (kvbT[(2 * j + i) * 128:(2 * j + i + 1) * 128] * SC_W).astype(F8)

    in_maps = []
    for c in range(NCORES):
        in_maps.append({
            "hsT": np.ascontiguousarray(hsT[:, c * TPC:(c + 1) * TPC]),
            "waT": waT,
            "qb8": qb8,
            "qbb": qbb,
            "kv8": kv8,
        })
    return in_maps


def kernel(hidden_states, q_a_w, q_b_w, kv_a_w, kv_b_w, q_a_ln_w, kv_a_ln_w,
           _trace=False):
    _ensure_env()
    from concourse.bass_utils import run_bass_kernel_spmd

    nc = _build()
    in_maps = _prep_inputs(hidden_states, q_a_w, kv_a_w, q_b_w, kv_b_w,
                           q_a_ln_w, kv_a_ln_w)
    res = run_bass_kernel_spmd(nc, in_maps, list(range(NCORES)), trace=_trace)

    out = np.empty((B, 3 * H, S, D_Q), dtype=np.float32)
    for c in range(NCORES):
        out[c // (S // TPC), :, (c % (S // TPC)) * TPC:((c % (S // TPC)) + 1) * TPC, :] = \
            res.results[c]["out"].astype(np.float32)
    out[:, 2 * H:, :, D_V:] = 0.0      # v padding is exact zeros
    if _trace:
        kernel.last_exec_time_ns = res.exec_time_ns
        kernel.last_results = res
    return out


# revision 13
# speedup vs baseline: 1.0985x; 1.0985x over previous
"""MLA QKV projection kernel for Trainium2 (8 NeuronCores, Bass/Tile).

Computes the DeepSeek-MLA q/k/v projection:
  q  = rmsnorm(hs @ q_a_w.T) @ q_b_w.T          -> [b, H, s, 192]  (pe cols rope-interleaved)
  ckv = hs @ kv_a_w.T ; compressed, k_pe = split
  kv = rmsnorm(compressed) @ kv_b_w.T           -> k_nope, v
  out = concat([q, concat(k_nope, k_pe), pad(v)], head axis)  -> [b, 3H, s, 192]

Strategy: pure data-parallel over tokens (8192 tokens -> 1024/core); weights
replicated, loaded from HBM exactly once per core. GEMMs run bf16 except the
kv-B projection (fp8e4 DoubleRow, K-paired: 2x rate) and the first 512 of the
q-B projection's K (also fp8e4 DoubleRow). The fp8/bf16 parts accumulate in a
common 2^14-scaled PSUM domain (acts x16 fp8 | weights x1024 fp8 | bf16
weights x16384), so every B eviction is one uniform 1/16384-scaled copy.
RMSNorm is applied by rescaling the A-GEMM output in SBUF (per-token scale =
per-partition broadcast); layernorm gammas and the rope interleave permutation
are folded into the weights on the host. Matmuls are grouped 4-consecutive
per stationary tile (512-col chunks of a [128,2048] PSUM tile) to amortize
the ~200ns LDWEIGHTS cost of switching the stationary operand. PSUM
evictions are split across the scalar/vector/pool engines and B weights are
prefetched a chunk-group ahead on a loads-only DMA queue so the PE never
stalls on PSUM rotation or weight arrival.
"""

import sys
import types

import numpy as np

# ---- constants (hardcoded problem shape) ----
H = 32
D_NOPE = 128
D_ROPE = 64
D_Q = 192
D_V = 128
R_KV = 512
RQ = 1536
DMODEL = 4096
EPS = 1e-6
B, S = 2, 4096
NTOK = B * S            # 8192
NCORES = 8
TPC = NTOK // NCORES    # 1024 tokens per core
KT = DMODEL // 128      # 32 k-tiles for the A GEMM
KSEG = 4                # A GEMM k-segments (SBUF-accumulated)
KPS = KT // KSEG        # 8 k-tiles per segment
ACOLS = RQ + R_KV       # 2048 (kpe's 64 cols handled in a separate pass)
QOUT = H * D_Q          # 6144
KVOUT = H * (D_NOPE + D_V)  # 8192

Q8_J = 3                # q-B: ktile pairs 0..Q8_J*2-1 in fp8 DR (768 of K)
QB_K = 12 - 2 * Q8_J    # remaining q-B bf16 ktiles
SC_ACT = 16.0           # fp8 activation scale
SC_W = 1024.0           # fp8 weight scale
SC_PSUM = SC_ACT * SC_W  # 16384 = common B psum domain


def _ensure_env():
    for p in ("/opt/trn_rl_repo", "/root/.axon_site"):
        if p not in sys.path:
            sys.path.insert(0, p)
    if "antenv.axon_hooks" not in sys.modules:
        try:
            import antenv  # noqa: F401
            import antenv.axon_hooks  # noqa: F401
        except ImportError:
            mod = types.ModuleType("antenv.axon_hooks")
            mod._hook = None
            mod.set_axon_ntff_profile_hook = lambda h: setattr(mod, "_hook", h)
            mod.get_axon_ntff_profile_hook = lambda: mod._hook
            sys.modules["antenv.axon_hooks"] = mod
            try:
                import antenv
                antenv.axon_hooks = mod
            except ImportError:
                pass


def _perm64():
    # inverse view of x.reshape(32,2).swapaxes ->  y[k] = x[2*(k%32) + k//32]
    return np.array([2 * (k % 32) + k // 32 for k in range(64)], dtype=np.int64)


_CACHE = {}


def _build():
    if "nc" in _CACHE:
        return _CACHE["nc"]
    _ensure_env()
    from concourse import bacc
    import concourse.mybir as mybir
    import concourse.tile as tile
    from concourse.masks import make_identity

    F32 = mybir.dt.float32
    BF16 = mybir.dt.bfloat16
    FP8 = mybir.dt.float8e4
    DR = mybir.MatmulPerfMode.DoubleRow
    AF = mybir.ActivationFunctionType
    ALU = mybir.AluOpType

    nc = bacc.Bacc("TRN2", target_bir_lowering=False, debug=False)
    hsT_d = nc.dram_tensor("hsT", [DMODEL, TPC], BF16, kind="ExternalInput")
    waT_d = nc.dram_tensor("waT", [DMODEL, ACOLS + D_ROPE], BF16, kind="ExternalInput")
    qb8_d = nc.dram_tensor("qb8", [Q8_J, 128, 2, QOUT], FP8, kind="ExternalInput")
    qbb_d = nc.dram_tensor("qbb", [QB_K * 128, QOUT], BF16, kind="ExternalInput")
    kv8_d = nc.dram_tensor("kv8", [2, 128, 2, KVOUT], FP8, kind="ExternalInput")
    out_d = nc.dram_tensor("out", [3 * H, TPC, D_Q], BF16, kind="ExternalOutput")

    def outv(h, mp, d0, w, n_m=2):
        # DRAM view [p, mi, w] for tokens mp*256..(mp+1)*256, head h cols d0:d0+w
        return out_d[h, mp * 128 * n_m:(mp + 1) * 128 * n_m, d0:d0 + w].rearrange(
            "(mi p) w -> p mi w", p=128)

    HALF = ACOLS // 2

    with tile.TileContext(nc) as tc:
        with tc.tile_pool(name="persist", bufs=1) as persist:
            kpe_sb = persist.tile([128, 8, D_ROPE], BF16)
            stats = persist.tile([128, 8, 2], F32)
            s_q = persist.tile([128, 8], F32)
            s_kv = persist.tile([128, 8], F32)
            at_q8 = persist.tile([128, Q8_J, 2, 8, 128], FP8)   # 4KB/p
            at_qb = persist.tile([128, QB_K, 8, 128], BF16)     # 16KB/p
            at_kv8 = persist.tile([128, 2, 2, 8, 128], FP8)     # 4KB/p
            sconst = persist.tile([128, 1], F32)
            ident = persist.tile([128, 128], F32)
            identb = persist.tile([128, 128], BF16)
            nc.gpsimd.memset(sconst, 1.0 / SC_PSUM)
            make_identity(nc, ident)
            nc.vector.tensor_copy(identb, ident)

            # ---------------- phases A + T (a_sb-scoped) ----------------
            with tc.tile_pool(name="aph", bufs=1) as aph:
                a_sb = aph.tile([128, 8, ACOLS], BF16)          # 32KB/p token-major A out
                kpeT = aph.tile([64, TPC], BF16)
                scr = aph.tile([128, RQ], F32)                  # square scratch
                scrb = aph.tile([128, 2, HALF], BF16)           # psum-evict staging

                with tc.tile_pool(name="hst", bufs=1) as hstp:
                    hst = [hstp.tile([128, TPC], BF16, name=f"hst{k}", tag=f"hst{k}")
                           for k in range(KT)]
                    for k in range(KT):
                        eng = nc.scalar if k % 2 == 0 else nc.gpsimd
                        eng.dma_start(out=hst[k], in_=hsT_d[k * 128:(k + 1) * 128, :])

                    with tc.tile_pool(name="wa", bufs=KPS + 2) as wap, \
                         tc.tile_pool(name="scl", bufs=4) as sclp, \
                         tc.tile_pool(name="psA", bufs=2, space="PSUM") as psA:
                        for kseg in range(KSEG):
                            wts = []
                            for kk in range(KPS):
                                k = kseg * KPS + kk
                                t = wap.tile([128, ACOLS], BF16, tag="wa")
                                nc.sync.dma_start(out=t, in_=waT_d[k * 128:(k + 1) * 128, 0:ACOLS])
                                wts.append(t)
                            for mp in range(4):
                                pss = [psA.tile([128, ACOLS], F32, tag="psA", name=f"psA{i}")
                                       for i in range(2)]
                                def a_mm(kk, mi):
                                    k = kseg * KPS + kk
                                    m = 2 * mp + mi
                                    for c in range(4):
                                        nc.tensor.matmul(
                                            pss[mi][:, c * 512:(c + 1) * 512],
                                            hst[k][:, m * 128:(m + 1) * 128],
                                            wts[kk][:, c * 512:(c + 1) * 512],
                                            start=(kk == 0), stop=(kk == KPS - 1))
                                for kk in range(KPS - 2):
                                    for mi in range(2):
                                        a_mm(kk, mi)
                                for mi in range(2):
                                    for kk in (KPS - 2, KPS - 1):
                                        a_mm(kk, mi)
                                for mi in range(2):
                                    m = 2 * mp + mi
                                    lo = a_sb[:, m, 0:HALF]
                                    hi = a_sb[:, m, HALF:ACOLS]
                                    if kseg == 0:
                                        nc.scalar.activation(lo, pss[mi][:, 0:HALF], AF.Copy)
                                        nc.vector.tensor_copy(hi, pss[mi][:, HALF:ACOLS])
                                    else:
                                        # DVE adds low half from PSUM; scalar stages the
                                        # high half to SBUF (fast PSUM release), pool adds it
                                        nc.vector.tensor_add(lo, lo, pss[mi][:, 0:HALF])
                                        nc.scalar.activation(scrb[:, mi, :], pss[mi][:, HALF:ACOLS], AF.Copy)
                                        nc.gpsimd.tensor_add(hi, hi, scrb[:, mi, :])
                                    if kseg == KSEG - 1:
                                        nc.scalar.activation(scr[:, 0:RQ], a_sb[:, m, 0:RQ],
                                                             AF.Square, accum_out=stats[:, m, 0:1])
                                        nc.scalar.activation(scr[:, 0:R_KV], a_sb[:, m, RQ:ACOLS],
                                                             AF.Square, accum_out=stats[:, m, 1:2])
                                if kseg == KSEG - 1:
                                    # per-pair rmsnorm scales + in-place rescale of a_sb
                                    for (dst, idx, dim) in ((s_q, 0, RQ), (s_kv, 1, R_KV)):
                                        t = sclp.tile([128, 2], F32, tag="sclt")
                                        nc.vector.tensor_scalar(
                                            out=t, in0=stats[:, 2 * mp:2 * mp + 2, idx],
                                            scalar1=1.0 / dim, scalar2=EPS,
                                            op0=ALU.mult, op1=ALU.add)
                                        nc.vector.reciprocal(t, t)
                                        nc.scalar.activation(dst[:, 2 * mp:2 * mp + 2], t, AF.Sqrt)
                                    for mi in range(2):
                                        m = 2 * mp + mi
                                        nc.scalar.activation(a_sb[:, m, 0:RQ], a_sb[:, m, 0:RQ],
                                                             AF.Copy, scale=s_q[:, m:m + 1])
                                        nc.vector.tensor_scalar_mul(
                                            a_sb[:, m, RQ:ACOLS], a_sb[:, m, RQ:ACOLS],
                                            s_kv[:, m:m + 1])

                    # ---- kpe pass: weights-stationary, output lands transposed ----
                    with tc.tile_pool(name="wk", bufs=4) as wkp, \
                         tc.tile_pool(name="psK", bufs=1, space="PSUM") as psKp:
                        psK = psKp.tile([64, TPC], F32)
                        for k in range(KT):
                            wk = wkp.tile([128, D_ROPE], BF16, tag="wk")
                            nc.sync.dma_start(out=wk, in_=waT_d[k * 128:(k + 1) * 128, ACOLS:ACOLS + D_ROPE])
                            for t2 in range(2):
                                nc.tensor.matmul(psK[:, t2 * 512:(t2 + 1) * 512], wk,
                                                 hst[k][:, t2 * 512:(t2 + 1) * 512],
                                                 start=(k == 0), stop=(k == KT - 1))
                        nc.vector.tensor_copy(kpeT, psK)

                # transpose kpe back to token-major; broadcast to all 32 key heads
                with tc.tile_pool(name="psKt", bufs=2, space="PSUM") as psKtp:
                    for m in range(8):
                        ptr = psKtp.tile([128, D_ROPE], BF16, tag="ptr")
                        nc.tensor.transpose(ptr, kpeT[:, m * 128:(m + 1) * 128], identb[0:64, 0:64])
                        nc.vector.tensor_copy(kpe_sb[:, m, :], ptr)
                for h in range(H):
                    eng = nc.gpsimd if h % 2 == 0 else nc.scalar
                    eng.dma_start(out=outv(H + h, 0, D_NOPE, D_ROPE, n_m=8),
                                  in_=kpe_sb[:, :, :])

                # ---------------- phase T: transpose a_sb -> at tiles ----------------
                with tc.tile_pool(name="psT", bufs=4, space="PSUM") as psTp:
                    for f in range(16):
                        pt = psTp.tile([128, 8, 128], BF16, tag="pt")
                        for m in range(8):
                            nc.tensor.transpose(pt[:, m, :], a_sb[:, m, f * 128:(f + 1) * 128], identb)
                        if f < 2 * Q8_J:
                            nc.vector.tensor_scalar_mul(at_q8[:, f // 2, f % 2], pt, SC_ACT)
                        elif f < 12:
                            nc.vector.tensor_copy(at_qb[:, f - 2 * Q8_J], pt)
                        else:
                            nc.vector.tensor_scalar_mul(at_kv8[:, (f - 12) // 2, (f - 12) % 2], pt, SC_ACT)

            # ---------------- phase B: the two B GEMMs ----------------
            with tc.tile_pool(name="wq8", bufs=2 * Q8_J + 2) as wq8p, \
                 tc.tile_pool(name="wqb", bufs=2 * QB_K + 1) as wqbp, \
                 tc.tile_pool(name="kv8", bufs=6) as kv8p, \
                 tc.tile_pool(name="ev", bufs=2) as evp, \
                 tc.tile_pool(name="evkv", bufs=2) as evkvp, \
                 tc.tile_pool(name="psB", bufs=2, space="PSUM") as psB:

                def load_q_w(cg):
                    w8ts, wbts = [], []
                    for j in range(Q8_J):
                        t = wq8p.tile([128, 2, 2048], FP8, tag="wq8", name=f"wq8_{cg}_{j}")
                        nc.sync.dma_start(out=t, in_=qb8_d[j, :, :, cg * 2048:(cg + 1) * 2048])
                        w8ts.append(t)
                    for kk in range(QB_K):
                        t = wqbp.tile([128, 2048], BF16, tag="wqb", name=f"wqb_{cg}_{kk}")
                        nc.sync.dma_start(out=t, in_=qbb_d[kk * 128:(kk + 1) * 128, cg * 2048:(cg + 1) * 2048])
                        wbts.append(t)
                    return w8ts, wbts

                def load_kv_w(cg):
                    kvts = []
                    for j in range(2):
                        t = kv8p.tile([128, 2, 2048], FP8, tag="kv8", name=f"kv8_{cg}_{j}")
                        nc.sync.dma_start(out=t, in_=kv8_d[j, :, :, cg * 2048:(cg + 1) * 2048])
                        kvts.append(t)
                    return kvts

                def q_evict_dma(pss, mp, cg):
                    ev = evp.tile([128, 2, 2048], BF16, tag="ev")
                    nc.vector.tensor_scalar_mul(ev[:, 0, :], pss[0], 1.0 / SC_PSUM)
                    nc.scalar.activation(ev[:, 1, :], pss[1], AF.Copy, scale=sconst[:, 0:1])
                    col = cg * 2048
                    end = col + 2048
                    while col < end:
                        h = col // D_Q
                        seg = min(end, (h + 1) * D_Q)
                        nc.gpsimd.dma_start(out=outv(h, mp, col - h * D_Q, seg - col),
                                            in_=ev[:, :, col - cg * 2048:seg - cg * 2048])
                        col = seg

                def kv_evict_dma(pss, mp, cg):
                    # [128, 2, 8 heads, 256] view; k_nope/v each go out in ONE 4D DMA
                    ev = evkvp.tile([128, 2, 8, 256], BF16, tag="evkv")
                    nc.vector.tensor_scalar_mul(ev[:, 0], pss[0], 1.0 / SC_PSUM)
                    nc.scalar.activation(ev[:, 1], pss[1], AF.Copy, scale=sconst[:, 0:1])
                    h0 = cg * 8
                    for mi in range(2):
                        t0 = mp * 256 + mi * 128
                        kn = out_d[H + h0:H + h0 + 8, t0:t0 + 128, 0:D_NOPE].rearrange(
                            "h p w -> p h w")
                        vv = out_d[2 * H + h0:2 * H + h0 + 8, t0:t0 + 128, 0:D_V].rearrange(
                            "h p w -> p h w")
                        nc.gpsimd.dma_start(out=kn, in_=ev[:, mi, :, 0:D_NOPE])
                        nc.gpsimd.dma_start(out=vv, in_=ev[:, mi, :, D_NOPE:256])

                # B-q: 3 chunk-groups of 2048 cols, weights prefetched one cg ahead
                qw = {0: load_q_w(0)}
                kvw = {}
                for cg in range(3):
                    w8ts, wbts = qw.pop(cg)
                    for mp in range(4):
                        pss = [psB.tile([128, 2048], F32, tag="psB", name=f"psB{i}")
                               for i in range(2)]
                        def q_dr_mm(j, mi):
                            m = 2 * mp + mi
                            for c in range(4):
                                nc.tensor.matmul(
                                    pss[mi][:, c * 512:(c + 1) * 512],
                                    at_q8[:, j, :, m, :],
                                    w8ts[j][:, :, c * 512:(c + 1) * 512],
                                    start=(j == 0), stop=False, perf_mode=DR)
                        def q_bf_mm(kk, mi):
                            m = 2 * mp + mi
                            for c in range(4):
                                nc.tensor.matmul(
                                    pss[mi][:, c * 512:(c + 1) * 512],
                                    at_qb[:, kk, m, :],
                                    wbts[kk][:, c * 512:(c + 1) * 512],
                                    start=False, stop=(kk == QB_K - 1))
                        for j in range(Q8_J):
                            for mi in range(2):
                                q_dr_mm(j, mi)
                        for kk in range(QB_K - 2):
                            for mi in range(2):
                                q_bf_mm(kk, mi)
                        for mi in range(2):
                            for kk in (QB_K - 2, QB_K - 1):
                                q_bf_mm(kk, mi)
                        if mp == 0:
                            if cg < 2:
                                qw[cg + 1] = load_q_w(cg + 1)
                            else:
                                kvw[0] = load_kv_w(0)
                                kvw[1] = load_kv_w(1)
                        q_evict_dma(pss, mp, cg)

                # B-kv: 4 chunk-groups of 2048 cols (8 half-heads each)
                for cg in range(4):
                    kvts = kvw.pop(cg)
                    for mp in range(4):
                        pss = [psB.tile([128, 2048], F32, tag="psB", name=f"psB{i}")
                               for i in range(2)]
                        for mi in range(2):
                            m = 2 * mp + mi
                            for j in range(2):
                                for c in range(4):
                                    nc.tensor.matmul(
                                        pss[mi][:, c * 512:(c + 1) * 512],
                                        at_kv8[:, j, :, m, :],
                                        kvts[j][:, :, c * 512:(c + 1) * 512],
                                        start=(j == 0), stop=(j == 1),
                                        perf_mode=DR)
                        if mp == 0 and cg + 2 < 4:
                            kvw[cg + 2] = load_kv_w(cg + 2)
                        kv_evict_dma(pss, mp, cg)

    nc.compile()
    _CACHE["nc"] = nc
    return nc


def _prep_inputs(hidden_states, q_a_w, kv_a_w, q_b_w, kv_b_w, q_a_ln_w, kv_a_ln_w):
    import ml_dtypes
    F8 = ml_dtypes.float8_e4m3
    BF = ml_dtypes.bfloat16
    f32 = np.float32
    hs = np.asarray(hidden_states, dtype=f32).reshape(NTOK, DMODEL)
    hsT = np.ascontiguousarray(hs.T).astype(BF)            # [4096, 8192]
    perm = _perm64()

    q_a_w = np.asarray(q_a_w, dtype=f32)
    kv_a_w = np.asarray(kv_a_w, dtype=f32)
    kv_a_pe = kv_a_w[R_KV:][perm]                          # de-interleave k_pe rows
    wa = np.concatenate([q_a_w, kv_a_w[:R_KV], kv_a_pe], axis=0)   # [2112, 4096]
    waT = np.ascontiguousarray(wa.T).astype(BF)            # [4096, 2112]

    qb = np.asarray(q_b_w, dtype=f32) * np.asarray(q_a_ln_w, dtype=f32)[None, :]
    qb = qb.reshape(H, D_Q, RQ).copy()
    qb[:, D_NOPE:, :] = qb[:, D_NOPE + perm, :]            # de-interleave q_pe rows
    qbT = np.ascontiguousarray(qb.reshape(QOUT, RQ).T)     # [1536, 6144] f32

    # fp8 DR part: ktile pairs j -> planes i hold ktiles 2j+i, scaled x1024
    qb8 = np.empty((Q8_J, 128, 2, QOUT), dtype=F8)
    for j in range(Q8_J):
        for i in range(2):
            qb8[j, :, i, :] = (qbT[(2 * j + i) * 128:(2 * j + i + 1) * 128] * SC_W).astype(F8)
    # bf16 part scaled x16384 into the common psum domain
    qbb = np.ascontiguousarray(qbT[2 * Q8_J * 128:] * SC_PSUM).astype(BF)

    kvb = np.asarray(kv_b_w, dtype=f32) * np.asarray(kv_a_ln_w, dtype=f32)[None, :]
    kvbT = np.ascontiguousarray(kvb.T)                     # [512, 8192] f32
    kv8 = np.empty((2, 128, 2, KVOUT), dtype=F8)
    for j in range(2):
        for i in range(2):
            kv8[j, :, i, :] = (kvbT[(2 * j + i) * 128:(2 * j + i + 1) * 128] * SC_W).astype(F8)

    in_maps = []
    for c in range(NCORES):
        in_maps.append({
            "hsT": np.ascontiguousarray(hsT[:, c * TPC:(c + 1) * TPC]),
            "waT": waT,
            "qb8": qb8,
            "qbb": qbb,
            "kv8": kv8,
        })
    return in_maps


def kernel(hidden_states, q_a_w, q_b_w, kv_a_w, kv_b_w, q_a_ln_w, kv_a_ln_w,
           _trace=False):
    _ensure_env()
    from concourse.bass_utils import run_bass_kernel_spmd

    nc = _build()
    in_maps = _prep_inputs(hidden_states, q_a_w, kv_a_w, q_b_w, kv_b_w,
                           q_a_ln_w, kv_a_ln_w)
    res = run_bass_kernel_spmd(nc, in_maps, list(range(NCORES)), trace=_trace)

    out = np.empty((B, 3 * H, S, D_Q), dtype=np.float32)
    for c in range(NCORES):
        out[c // (S // TPC), :, (c % (S // TPC)) * TPC:((c % (S // TPC)) + 1) * TPC, :] = \
            res.results[c]["out"].astype(np.float32)
    out[:, 2 * H:, :, D_V:] = 0.0      # v padding is exact zeros
    if _trace:
        kernel.last_exec_time_ns = res.exec_time_ns
        kernel.last_results = res
    return out
